# revision 2
# baseline (speedup 1.0000x reference)
"""GroupQueryAttention kernel for 8 Trainium2 NeuronCores.

Problem: B=2, S=2048, E=2048, H=16 heads, G=4 kv-groups, head_dim=128.

Sharding: batch x kv-group. Core d owns batch b=d//4 and kv-group g=d%4,
i.e. the 4 query heads of that group (512-column slice of Wq, 128-column
slice of Wk/Wv, 512-row slice of Wo). Each core produces a partial
y^T[E,S] for its batch in bf16; the host sums the 4 group-partials per
batch, rescales, adds bo, and transposes back.

Precision scheme (validated to rel_err ~1.67e-2 vs f32 reference,
budget 2e-2):
  - Projections (Q,K,V) and the Wo matmul run on the PE in fp8-e4m3
    DoubleRow perf mode with an exact hi+lo decomposition: operand
    a ~= a_hi + a_lo where a_hi = e4m3(s*a) and a_lo = e4m3(s*a - a_hi).
    The product W@x = Wh@xh + Wl@xh + Wh@xl needs 3 fp8 tile-products
    per e-tile; DoubleRow packs 2 tile-products per instruction at 0.5
    cycles/row, i.e. 0.75 of the bf16 cost at near-bf16 accuracy.
    Power-of-2 scales keep operands out of e4m3's subnormal range
    (x*16, W*64, v*32); descales fold into the PSUM-evacuation copies
    and the host-side epilogue.
  - q/k are stored fp16 and scores run fp16 at 1 cycle/row (fp8
    anywhere in the scores path costs >2e-2 rel err).
  - exp writes UNNORMALIZED attention weights directly as e4m3 scaled
    by 1/2 (via the exp bias: exp(s + ln .5)); the scale cancels in
    otc = pso/den. AV then runs DoubleRow with v as an exact fp8
    hi/lo pair against the fp8 attn (the single e4m3 quantization of
    attn is the dominant error term, ~1.65e-2). The softmax
    denominator is an fp8-ones DoubleRow matmul accumulated with AV's
    operands, so numerator and denominator see identical quantization.
  - Softmax skips max-subtraction (scores are O(1) by construction) and
    the K bias (a per-query constant shift cancels in softmax).

Schedule: phase A streams K/V/Q projections over s-chunks (previous
chunk's V/Q ahead of the current chunk's K so x DMAs stay hidden); then
16 attention slots (head h, 512-wide q-chunk qc): 8 score-pairs -> exp
(fp8 attn) -> den ones-matmul + AV on PE -> reciprocal (DVE) ->
otc split to fp8 hi/lo (DVE+gpsimd). Wo of q-chunk qc-1 interleaves
into qc's slots as PE filler; the last chunk's V/Q fill qc0's slots.
"""

import math

import numpy as np

B = 2
S = 2048
E = 2048
HD = 128
HLOC = 4  # heads per core (= one kv group)
NCORES = 8
ECH = E // 128  # 16 e-tiles for contraction
QC = 512  # q-chunk width in attention
NQC = S // QC  # 4
KJT = S // 128  # 16 kj tiles
INV_SQRT_HD = 1.0 / math.sqrt(HD)

XS = 16.0  # x fp8 scale
WS = 64.0  # weight fp8 scale
VS = 32.0  # v fp8 scale (folds the otc fp8 scale in via the AV matmul)
AS = 0.5  # attn fp8 scale, applied inside exp; cancels in pso/den
LN_AS = math.log(AS)
ISC = 1.0 / (XS * WS)  # q/k descale
ISCV = VS / (XS * WS)  # v descale (+ otc prescale)
YDS = VS * WS  # host-side y descale

# phase-A chunks: two 128-wide starter chunks for an early PE start
CHUNKS = [(0, 128), (128, 128)] + [(256 * i, 256) for i in range(1, S // 256)]

_CACHE = {}


def _build():
    import concourse.bacc as bacc
    import concourse.mybir as mybir
    import concourse.tile as tile

    f32 = mybir.dt.float32
    f16 = mybir.dt.float16
    bf16 = mybir.dt.bfloat16
    e4 = mybir.dt.float8e4
    AF = mybir.ActivationFunctionType
    ALU = mybir.AluOpType
    DR = mybir.MatmulPerfMode.DoubleRow

    nc = bacc.Bacc("TRN2", target_bir_lowering=False, debug=False)

    xh_d = nc.dram_tensor("xh", [128, ECH * S], e4, kind="ExternalInput").ap()
    xl_d = nc.dram_tensor("xl", [128, ECH * S], e4, kind="ExternalInput").ap()
    wq_d = nc.dram_tensor("wq", [HLOC, 128, 2, ECH, HD], e4, kind="ExternalInput").ap()
    bq_d = nc.dram_tensor("bq", [HLOC * HD], f32, kind="ExternalInput").ap()
    wk_d = nc.dram_tensor("wk", [128, 2, ECH, HD], e4, kind="ExternalInput").ap()
    wv_d = nc.dram_tensor("wv", [128, 2, ECH, HD], e4, kind="ExternalInput").ap()
    bvb_d = nc.dram_tensor("bvb", [128, HD], f16, kind="ExternalInput").ap()
    wo_d = nc.dram_tensor("wo", [128, HLOC, 2, E], e4, kind="ExternalInput").ap()
    yT = nc.dram_tensor("yT", [E, S], bf16, kind="ExternalOutput").ap()

    with tile.TileContext(nc) as tc:
        with (
            tc.tile_pool(name="pers", bufs=1) as pers,
            tc.tile_pool(name="kv", bufs=1) as kvp,
            tc.tile_pool(name="xt", bufs=1) as xpool,
            tc.tile_pool(name="attn", bufs=2) as apool,
            tc.tile_pool(name="tree", bufs=2) as tpool,
            tc.tile_pool(name="soft", bufs=2) as spool,
            tc.tile_pool(name="otc", bufs=3) as opool,
            tc.tile_pool(name="yb", bufs=4) as ypool,
            tc.tile_pool(name="ps_proj", bufs=2, space="PSUM") as pp,
            tc.tile_pool(name="ps_sc", bufs=2, space="PSUM") as psc,
            tc.tile_pool(name="ps_o", bufs=1, space="PSUM") as po,
        ):
            def xslice(d, s0, w):
                return d[:, ECH * s0 : ECH * (s0 + w)]

            # --- startup DMAs, split hi/lo and ordered so the first DR
            # matmuls (which need only the hi halves) start earliest ---
            wk_sb = pers.tile([128, 2, ECH, HD], e4)
            nc.sync.dma_start(out=wk_sb[:, 0], in_=wk_d[:, 0])

            xt_t = {}

            def xdma(ci):
                # all chunks stay resident: phase-B Q fillers read them late
                s0, w = CHUNKS[ci]
                th = xpool.tile([128, ECH, w], e4, tag=f"xh{ci}", name=f"xh{ci}")
                nc.sync.dma_start(
                    out=th.rearrange("p a b -> p (a b)"), in_=xslice(xh_d, s0, w)
                )
                tl = xpool.tile([128, ECH, w], e4, tag=f"xl{ci}", name=f"xl{ci}")
                nc.sync.dma_start(
                    out=tl.rearrange("p a b -> p (a b)"), in_=xslice(xl_d, s0, w)
                )
                xt_t[ci] = (th, tl)

            xdma(0)
            nc.sync.dma_start(out=wk_sb[:, 1], in_=wk_d[:, 1])
            xdma(1)
            wv_sb = pers.tile([128, 2, ECH, HD], e4)
            nc.sync.dma_start(out=wv_sb, in_=wv_d)
            bvb_sb = pers.tile([128, HD], f16)
            nc.sync.dma_start(out=bvb_sb, in_=bvb_d)
            xdma(2)
            wq_h = []
            for h in range(HLOC):
                wqh = pers.tile([128, 2, ECH, HD], e4, tag=f"wqh{h}", name=f"wqh{h}")
                wq_h.append(wqh)
            nc.sync.dma_start(out=wq_h[0], in_=wq_d[0])
            bq_sb = pers.tile([128, HLOC], f32)
            nc.sync.dma_start(out=bq_sb, in_=bq_d.rearrange("(h d) -> d h", d=128))
            xdma(3)
            nc.sync.dma_start(out=wq_h[1], in_=wq_d[1])
            xdma(4)
            nc.sync.dma_start(out=wq_h[2], in_=wq_d[2])
            xdma(5)
            nc.sync.dma_start(out=wq_h[3], in_=wq_d[3])
            wo_sb = pers.tile([128, HLOC, 2, E], e4)
            ones8 = pers.tile([128, 2, 128], e4)
            nc.vector.memset(ones8, 1.0)
            lnas_sb = pers.tile([128, 1], f32)
            nc.vector.memset(lnas_sb, LN_AS)

            # per-batch activations (one batch per core)
            qt_sb = kvp.tile([128, HLOC, S], f16)
            kt_sb = kvp.tile([128, S], f16)
            v8_sb = kvp.tile([128, KJT, 2, HD], e4)

            # --- S3 fp8 DoubleRow contraction: W@x = Wh@xh + Wl@xh + Wh@xl,
            # 3 DR instructions per pair of e-tiles, product-major order so
            # the first wave needs only the earliest DMAs ---
            def s3_matmuls(ps, w_sb, xht, xlt, c0=None, vmode=False):
                n = ECH // 2
                plan = [(0, xht), (0, xlt), (1, xht)]
                for wi, (half, xt) in enumerate(plan):
                    for p in range(n):
                        t = slice(2 * p, 2 * p + 2)
                        first = wi == 0 and p == 0
                        last = wi == 2 and p == n - 1
                        if vmode:
                            nc.tensor.matmul(
                                ps, lhsT=xt[:, t, c0 : c0 + 128],
                                rhs=w_sb[:, half, t, :],
                                start=first, stop=last, perf_mode=DR,
                            )
                        else:
                            nc.tensor.matmul(
                                ps, lhsT=w_sb[:, half, t, :], rhs=xt[:, t, :],
                                start=first, stop=last, perf_mode=DR,
                            )

            def k_unit(s0, w, xht, xlt):
                ps = pp.tile([128, w], f32, tag="ps_proj", name="ps")
                s3_matmuls(ps, wk_sb, xht, xlt)
                # no K bias: a per-query constant in the scores cancels
                nc.scalar.activation(kt_sb[:, s0 : s0 + w], ps, AF.Identity,
                                     scale=ISC)

            def q_unit(s0, w, xht, xlt, h):
                ps = pp.tile([128, w], f32, tag="ps_proj", name="ps")
                s3_matmuls(ps, wq_h[h], xht, xlt)
                # on DVE, not Act: an Act copy between exps delays the
                # score-psum ring in phase B
                nc.vector.scalar_tensor_tensor(
                    qt_sb[:, h, s0 : s0 + w], ps, ISC,
                    bq_sb[:, h : h + 1].broadcast_to((128, w)),
                    op0=ALU.mult, op1=ALU.add,
                )

            def v_unit(kj, s0, xht, xlt):
                # x is stationary (psum in [kpos, hd] layout); v then split
                # to an exact fp8 hi/lo pair for the DoubleRow AV matmul
                psv = pp.tile([128, HD], f32, tag="ps_proj", name="psv")
                s3_matmuls(psv, wv_sb, xht, xlt, c0=kj * 128 - s0, vmode=True)
                v16 = tpool.tile([128, HD], f16, tag="v16")
                nc.vector.scalar_tensor_tensor(
                    v16, psv, ISCV, bvb_sb, op0=ALU.mult, op1=ALU.add
                )
                nc.gpsimd.tensor_copy(v8_sb[:, kj, 0, :], v16)
                nc.gpsimd.tensor_tensor(
                    v8_sb[:, kj, 1, :], v16, v8_sb[:, kj, 0, :],
                    op=ALU.subtract,
                )

            NCH = len(CHUNKS)

            # --- attention slot pieces ---
            def score_pair(attn, h, q0, ktp):
                pss = psc.tile([128, 2, QC], f32, tag="ps_sc")
                for j in range(2):
                    kt = 2 * ktp + j
                    nc.tensor.matmul(
                        pss[:, j, :],
                        lhsT=kt_sb[:, kt * 128 : (kt + 1) * 128],
                        rhs=qt_sb[:, h, q0 : q0 + QC],
                        start=True,
                        stop=True,
                    )
                nc.scalar.activation(
                    attn[:, 2 * ktp : 2 * ktp + 2, :],
                    pss,
                    AF.Exp,
                    scale=INV_SQRT_HD,
                    bias=lnas_sb[:, 0:1],
                )

            def slot_post(attn, otc8, h, fillers, last=False):
                # denominator: fp8-ones DoubleRow matmul (matches the
                # quantized numerator exactly); AV DoubleRow with the exact
                # v hi/lo pair; both on the PE
                psd = po.tile([128, QC], f32, tag="ps_d", name="psd")
                for tp in range(KJT // 2):
                    nc.tensor.matmul(
                        psd, lhsT=ones8,
                        rhs=attn[:, 2 * tp : 2 * tp + 2, :],
                        start=(tp == 0), stop=(tp == KJT // 2 - 1),
                        perf_mode=DR,
                    )
                pso = po.tile([128, QC], f32, tag="ps_o")
                for kt in range(KJT):
                    rhs = attn[:, kt, :].unsqueeze(1).broadcast_to((128, 2, QC))
                    nc.tensor.matmul(
                        pso, lhsT=v8_sb[:, kt, :, :], rhs=rhs,
                        start=(kt == 0), stop=(kt == KJT - 1),
                        perf_mode=DR,
                    )
                rec = spool.tile([128, QC], f32, tag="rec")
                nc.vector.reciprocal(rec, psd)
                while fillers:
                    fillers.pop(0)()
                otc16 = tpool.tile([128, QC], f16, tag="otc16")
                nc.vector.tensor_tensor(otc16, pso, rec, op=ALU.mult)
                # DVE is faster; use it for the last slot where the otc
                # chain is on the critical path to the final Wo wave
                eng = nc.vector if last else nc.gpsimd
                eng.tensor_copy(otc8[:, h, 0, :], otc16)
                eng.tensor_tensor(
                    otc8[:, h, 1, :], otc16, otc8[:, h, 0, :],
                    op=ALU.subtract,
                )

            # --- Phase A: K/V projections chunk-streamed, with slot
            # (h0, qc0)'s score pairs folded in as kt slices land (score
            # pair ktp only needs kt up to chunk ktp+1) ---
            otc8_q = {}
            attn0 = apool.tile([128, KJT, QC], e4, tag="attn", name="attn0")
            attn1 = apool.tile([128, KJT, QC], e4, tag="attn", name="attn1")
            # x-independent extras per iteration: (head, chunk) Q units,
            # placed after the wq DMAs land
            EXTRA_Q = {
                3: [(0, 0)], 4: [(0, 1)], 5: [(0, 2), (1, 0)],
                6: [(1, 1), (2, 0)], 7: [(1, 2), (2, 1)], 8: [(2, 2)],
            }
            next_pair = 0
            for ci, (s0, w) in enumerate(CHUNKS):
                if ci not in xt_t:
                    xdma(ci)
                if ci == 8:
                    # h1's first 7 score pairs are x8-independent: they
                    # cover the x7/x8 DMA tail
                    for p in range(KJT // 2 - 1):
                        score_pair(attn1, 1, 0, p)
                k_unit(s0, w, xht=xt_t[ci][0], xlt=xt_t[ci][1])
                for kj in range(s0 // 128, (s0 + w) // 128):
                    v_unit(kj, s0, xt_t[ci][0], xt_t[ci][1])
                for qh, qci in EXTRA_Q.get(ci, []):
                    qs0, qw = CHUNKS[qci]
                    q_unit(qs0, qw, xt_t[qci][0], xt_t[qci][1], qh)
                if ci >= 5:
                    while next_pair < KJT // 2 and next_pair + 1 <= ci:
                        score_pair(attn0, 0, 0, next_pair)
                        next_pair += 1
            nc.sync.dma_start(out=wo_sb, in_=wo_d)
            score_pair(attn1, 1, 0, KJT // 2 - 1)
            otc8_0 = opool.tile([128, HLOC, 2, QC], e4, tag="otc8")
            otc8_q[0] = otc8_0
            slot_post(attn0, otc8_0, 0, [])
            slot_post(attn1, otc8_0, 1, [])

            # --- Phase B: remaining slots + Wo interleave ---
            def wo_unit(qc, ec, otc8, copy_act=False, alt_pool=0):
                if alt_pool == 1:
                    psy = po.tile([128, QC], f32, tag="ps_o", name="psy")
                elif alt_pool == 2:
                    psy = po.tile([128, QC], f32, tag="ps_d", name="psy")
                else:
                    psy = pp.tile([128, QC], f32, tag="ps_proj", name="psy")
                esl = slice(ec * 128, (ec + 1) * 128)
                for h in range(HLOC):
                    rhs = otc8[:, h, 0, :].unsqueeze(1).broadcast_to((128, 2, QC))
                    nc.tensor.matmul(psy, lhsT=wo_sb[:, h, :, esl], rhs=rhs,
                                     start=(h == 0), stop=False, perf_mode=DR)
                for j in range(2):
                    nc.tensor.matmul(
                        psy,
                        lhsT=wo_sb[:, 2 * j : 2 * j + 2, 0, esl],
                        rhs=otc8[:, 2 * j : 2 * j + 2, 1, :],
                        start=False, stop=(j == 1), perf_mode=DR,
                    )
                ybuf = ypool.tile([128, QC], bf16, tag="yb", name="ybuf")
                # gpsimd can't read PSUM; DVE does the y copies (Act is
                # saturated by exp in phase B, but idle in the final wave)
                if copy_act:
                    nc.scalar.copy(ybuf, psy)
                else:
                    nc.vector.tensor_copy(ybuf, psy)
                nc.sync.dma_start(
                    out=yT[esl, qc * QC : (qc + 1) * QC], in_=ybuf
                )

            def q_filler(h, ci):
                s0, w = CHUNKS[ci]
                return lambda: q_unit(s0, w, xt_t[ci][0], xt_t[ci][1], h)

            # per-slot PE filler plan; Q(h, chunks) for a q-chunk must land
            # in an earlier slot than the (h, qc) slot that reads it.
            # qc0 reads chunks 0-2, qc1: 3-4, qc2: 5-6, qc3: 7-8.
            FILLER_PLAN = {
                (0, 2): [(3, 0), (3, 1), (3, 2)],
                (0, 3): [(0, 3), (0, 4), (1, 3), (1, 4)],
                (1, 0): [(2, 3), (2, 4)],
                (1, 1): [(3, 3), (3, 4)],
                (1, 2): [(0, 5), (0, 6)],
                (1, 3): [(1, 5), (1, 6)],
                (2, 0): [(2, 5), (2, 6)],
                (2, 1): [(3, 5), (3, 6)],
                (2, 2): [(0, 7), (0, 8)],
                (2, 3): [(1, 7), (1, 8)],
                (3, 0): [(2, 7), (2, 8)],
                (3, 1): [(3, 7), (3, 8)],
            }
            for qc in range(NQC):
                q0 = qc * QC
                if qc == 0:
                    otc8 = otc8_q[0]
                else:
                    otc8 = opool.tile([128, HLOC, 2, QC], e4, tag="otc8")
                    otc8_q[qc % 2] = otc8
                for h in range(HLOC):
                    if qc == 0 and h <= 1:
                        continue  # folded into phase A
                    fillers = []
                    if qc > 0:
                        for i in range(4):
                            fillers.append(
                                lambda ec=h * 4 + i, o=otc8_q[(qc - 1) % 2]:
                                    wo_unit(qc - 1, ec, o)
                            )
                    for qh, qci in FILLER_PLAN.get((qc, h), []):
                        fillers.append(q_filler(qh, qci))
                    attn = apool.tile([128, KJT, QC], e4, tag="attn")
                    for ktp in range(KJT // 2):
                        score_pair(attn, h, q0, ktp)
                        if ktp % 2 == 1 and fillers:
                            fillers.pop(0)()
                    slot_post(attn, otc8, h, fillers,
                              last=(qc == NQC - 1 and h == HLOC - 1))
            for ec in range(ECH):
                wo_unit(NQC - 1, ec, otc8_q[(NQC - 1) % 2],
                        copy_act=(ec % 2 == 1),
                        alt_pool=(0, 1, 0, 2)[ec % 4])
    nc.finalize()
    return nc


def _get_nc():
    if "nc" not in _CACHE:
        _CACHE["nc"] = _build()
    return _CACHE["nc"]


def _hilo(a, s):
    import ml_dtypes

    E4 = ml_dtypes.float8_e4m3
    scaled = np.asarray(a, np.float32) * s
    hi = scaled.astype(E4)
    lo = (scaled - hi.astype(np.float32)).astype(E4)
    return hi, lo


def _shard_inputs(x, Wq, bq, Wk, bk, Wv, bv, Wo, bo):
    xT = np.asarray(x).transpose(0, 2, 1).astype(np.float32)  # [B, E, S]

    def xchunks(a):  # [E, S] fp8 -> [128, ECH*S] chunk-major
        parts = [
            a[:, s0 : s0 + w]
            .reshape(ECH, 128, w)
            .transpose(1, 0, 2)
            .reshape(128, ECH * w)
            for s0, w in CHUNKS
        ]
        return np.ascontiguousarray(np.concatenate(parts, axis=1))

    xcs = []
    for b in range(B):
        hi, lo = _hilo(xT[b], XS)
        xcs.append((xchunks(hi), xchunks(lo)))

    in_maps = []
    for d in range(NCORES):
        b = d // 4
        g = d % 4
        qh, ql = _hilo(Wq[:, g * 512 : (g + 1) * 512], WS)  # [E, 512]

        def wq_lay(a):
            return a.reshape(ECH, 128, HLOC, HD).transpose(2, 1, 0, 3)

        kh, kl = _hilo(Wk[:, g * 128 : (g + 1) * 128], WS)
        vh, vl = _hilo(Wv[:, g * 128 : (g + 1) * 128], WS)

        def wkv_lay(a):
            return a.reshape(ECH, 128, HD).transpose(1, 0, 2)

        oh, ol = _hilo(Wo[g * 512 : (g + 1) * 512, :], WS)  # [512, E]

        def wo_lay(a):
            return a.reshape(HLOC, 128, E).transpose(1, 0, 2)

        in_maps.append(
            {
                "xh": xcs[b][0],
                "xl": xcs[b][1],
                "wq": np.ascontiguousarray(
                    np.stack([wq_lay(qh), wq_lay(ql)], axis=2)
                ),
                "bq": np.ascontiguousarray(np.asarray(bq)[g * 512 : (g + 1) * 512]),
                "wk": np.ascontiguousarray(
                    np.stack([wkv_lay(kh), wkv_lay(kl)], axis=1)
                ),
                "wv": np.ascontiguousarray(
                    np.stack([wkv_lay(vh), wkv_lay(vl)], axis=1)
                ),
                "bvb": np.ascontiguousarray(
                    np.tile(
                        (np.asarray(bv)[g * 128 : (g + 1) * 128] * VS)[None, :],
                        (128, 1),
                    )
                ).astype(np.float16),
                "wo": np.ascontiguousarray(
                    np.stack([wo_lay(oh), wo_lay(ol)], axis=2)
                ),
            }
        )
    return in_maps


def _unshard(results, bo):
    acc = np.zeros((B, E, S), dtype=np.float32)
    for d, r in enumerate(results):
        acc[d // 4] += r["yT"].astype(np.float32)
    y = acc.transpose(0, 2, 1) / YDS + bo[None, None, :]
    return np.ascontiguousarray(y.astype(np.float32))


def kernel(x, Wq, bq, Wk, bk, Wv, bv, Wo, bo, **_):
    from concourse.bass_utils import run_bass_kernel_spmd

    nc = _get_nc()
    in_maps = _shard_inputs(x, Wq, bq, Wk, bk, Wv, bv, Wo, bo)
    res = run_bass_kernel_spmd(nc, in_maps, list(range(NCORES)))
    return _unshard(res.results, np.asarray(bo))


# revision 3
# speedup vs baseline: 1.0044x; 1.0044x over previous
"""GroupQueryAttention kernel for 8 Trainium2 NeuronCores.

Problem: B=2, S=2048, E=2048, H=16 heads, G=4 kv-groups, head_dim=128.

Sharding: batch x kv-group. Core d owns batch b=d//4 and kv-group g=d%4,
i.e. the 4 query heads of that group (512-column slice of Wq, 128-column
slice of Wk/Wv, 512-row slice of Wo). Each core produces a partial
y^T[E,S] for its batch in bf16; the host sums the 4 group-partials per
batch, rescales, adds bo, and transposes back.

Precision scheme (validated to rel_err ~1.67e-2 vs f32 reference,
budget 2e-2):
  - Projections (Q,K,V) and the Wo matmul run on the PE in fp8-e4m3
    DoubleRow perf mode with an exact hi+lo decomposition: operand
    a ~= a_hi + a_lo where a_hi = e4m3(s*a) and a_lo = e4m3(s*a - a_hi).
    The product W@x = Wh@xh + Wl@xh + Wh@xl needs 3 fp8 tile-products
    per e-tile; DoubleRow packs 2 tile-products per instruction at 0.5
    cycles/row, i.e. 0.75 of the bf16 cost at near-bf16 accuracy.
    Power-of-2 scales keep operands out of e4m3's subnormal range
    (x*16, W*64, v*32); descales fold into the PSUM-evacuation copies
    and the host-side epilogue.
  - q/k are stored fp16 and scores run fp16 at 1 cycle/row (fp8
    anywhere in the scores path costs >2e-2 rel err).
  - exp writes UNNORMALIZED attention weights directly as e4m3 scaled
    by 1/2 (via the exp bias: exp(s + ln .5)); the scale cancels in
    otc = pso/den. AV then runs DoubleRow with v as an exact fp8
    hi/lo pair against the fp8 attn (the single e4m3 quantization of
    attn is the dominant error term, ~1.65e-2). The softmax
    denominator is an fp8-ones DoubleRow matmul accumulated with AV's
    operands, so numerator and denominator see identical quantization.
  - Softmax skips max-subtraction (scores are O(1) by construction) and
    the K bias (a per-query constant shift cancels in softmax).

Schedule: phase A streams K/V/Q projections over s-chunks (previous
chunk's V/Q ahead of the current chunk's K so x DMAs stay hidden); then
16 attention slots (head h, 512-wide q-chunk qc): 8 score-pairs -> exp
(fp8 attn) -> den ones-matmul + AV on PE -> reciprocal (DVE) ->
otc split to fp8 hi/lo (DVE+gpsimd). Wo of q-chunk qc-1 interleaves
into qc's slots as PE filler; the last chunk's V/Q fill qc0's slots.
"""

import math

import numpy as np

B = 2
S = 2048
E = 2048
HD = 128
HLOC = 4  # heads per core (= one kv group)
NCORES = 8
ECH = E // 128  # 16 e-tiles for contraction
QC = 512  # q-chunk width in attention
NQC = S // QC  # 4
KJT = S // 128  # 16 kj tiles
INV_SQRT_HD = 1.0 / math.sqrt(HD)

XS = 16.0  # x fp8 scale
WS = 64.0  # weight fp8 scale
VS = 32.0  # v fp8 scale (folds the otc fp8 scale in via the AV matmul)
AS = 0.5  # attn fp8 scale, applied inside exp; cancels in pso/den
LN_AS = math.log(AS)
ISC = 1.0 / (XS * WS)  # q/k descale
ISCV = VS / (XS * WS)  # v descale (+ otc prescale)
YDS = VS * WS  # host-side y descale

# phase-A chunks: two 128-wide starter chunks for an early PE start
CHUNKS = [(0, 128), (128, 128)] + [(256 * i, 256) for i in range(1, S // 256)]

_CACHE = {}


def _build():
    import concourse.bacc as bacc
    import concourse.mybir as mybir
    import concourse.tile as tile

    f32 = mybir.dt.float32
    f16 = mybir.dt.float16
    bf16 = mybir.dt.bfloat16
    e4 = mybir.dt.float8e4
    AF = mybir.ActivationFunctionType
    ALU = mybir.AluOpType
    DR = mybir.MatmulPerfMode.DoubleRow

    nc = bacc.Bacc("TRN2", target_bir_lowering=False, debug=False)

    xh_d = nc.dram_tensor("xh", [128, ECH * S], e4, kind="ExternalInput").ap()
    xl_d = nc.dram_tensor("xl", [128, ECH * S], e4, kind="ExternalInput").ap()
    wq_d = nc.dram_tensor("wq", [HLOC, 128, 2, ECH, HD], e4, kind="ExternalInput").ap()
    bq_d = nc.dram_tensor("bq", [HLOC * HD], f32, kind="ExternalInput").ap()
    wk_d = nc.dram_tensor("wk", [128, 2, ECH, HD], e4, kind="ExternalInput").ap()
    wv_d = nc.dram_tensor("wv", [128, 2, ECH, HD], e4, kind="ExternalInput").ap()
    bvb_d = nc.dram_tensor("bvb", [128, HD], f16, kind="ExternalInput").ap()
    wo_d = nc.dram_tensor("wo", [128, HLOC, 2, E], e4, kind="ExternalInput").ap()
    yT = nc.dram_tensor("yT", [E, S], bf16, kind="ExternalOutput").ap()

    with tile.TileContext(nc) as tc:
        with (
            tc.tile_pool(name="pers", bufs=1) as pers,
            tc.tile_pool(name="kv", bufs=1) as kvp,
            tc.tile_pool(name="xt", bufs=1) as xpool,
            tc.tile_pool(name="attn", bufs=2) as apool,
            tc.tile_pool(name="tree", bufs=2) as tpool,
            tc.tile_pool(name="soft", bufs=2) as spool,
            tc.tile_pool(name="otc", bufs=3) as opool,
            tc.tile_pool(name="yb", bufs=4) as ypool,
            tc.tile_pool(name="ps_proj", bufs=2, space="PSUM") as pp,
            tc.tile_pool(name="ps_sc", bufs=2, space="PSUM") as psc,
            tc.tile_pool(name="ps_o", bufs=1, space="PSUM") as po,
        ):
            def xslice(d, s0, w):
                return d[:, ECH * s0 : ECH * (s0 + w)]

            # --- startup DMAs, split hi/lo and ordered so the first DR
            # matmuls (which need only the hi halves) start earliest ---
            wk_sb = pers.tile([128, 2, ECH, HD], e4)
            nc.sync.dma_start(out=wk_sb[:, 0], in_=wk_d[:, 0])

            xt_t = {}

            def xdma(ci):
                # all chunks stay resident: phase-B Q fillers read them late
                s0, w = CHUNKS[ci]
                th = xpool.tile([128, ECH, w], e4, tag=f"xh{ci}", name=f"xh{ci}")
                nc.sync.dma_start(
                    out=th.rearrange("p a b -> p (a b)"), in_=xslice(xh_d, s0, w)
                )
                tl = xpool.tile([128, ECH, w], e4, tag=f"xl{ci}", name=f"xl{ci}")
                nc.sync.dma_start(
                    out=tl.rearrange("p a b -> p (a b)"), in_=xslice(xl_d, s0, w)
                )
                xt_t[ci] = (th, tl)

            xdma(0)
            nc.sync.dma_start(out=wk_sb[:, 1], in_=wk_d[:, 1])
            xdma(1)
            wv_sb = pers.tile([128, 2, ECH, HD], e4)
            nc.sync.dma_start(out=wv_sb, in_=wv_d)
            bvb_sb = pers.tile([128, HD], f16)
            nc.sync.dma_start(out=bvb_sb, in_=bvb_d)
            xdma(2)
            wq_h = []
            for h in range(HLOC):
                wqh = pers.tile([128, 2, ECH, HD], e4, tag=f"wqh{h}", name=f"wqh{h}")
                wq_h.append(wqh)
            nc.sync.dma_start(out=wq_h[0], in_=wq_d[0])
            bq_sb = pers.tile([128, HLOC], f32)
            nc.sync.dma_start(out=bq_sb, in_=bq_d.rearrange("(h d) -> d h", d=128))
            xdma(3)
            nc.sync.dma_start(out=wq_h[1], in_=wq_d[1])
            xdma(4)
            nc.sync.dma_start(out=wq_h[2], in_=wq_d[2])
            xdma(5)
            nc.sync.dma_start(out=wq_h[3], in_=wq_d[3])
            wo_sb = pers.tile([128, HLOC, 2, E], e4)
            ones8 = pers.tile([128, 2, 128], e4)
            nc.vector.memset(ones8, 1.0)
            lnas_sb = pers.tile([128, 1], f32)
            nc.vector.memset(lnas_sb, LN_AS)

            # per-batch activations (one batch per core)
            qt_sb = kvp.tile([128, HLOC, S], f16)
            kt_sb = kvp.tile([128, S], f16)
            v8_sb = kvp.tile([128, KJT, 2, HD], e4)

            # --- S3 fp8 DoubleRow contraction: W@x = Wh@xh + Wl@xh + Wh@xl,
            # 3 DR instructions per pair of e-tiles, product-major order so
            # the first wave needs only the earliest DMAs ---
            def s3_matmuls(ps, w_sb, xht, xlt, c0=None, vmode=False):
                n = ECH // 2
                plan = [(0, xht), (0, xlt), (1, xht)]
                for wi, (half, xt) in enumerate(plan):
                    for p in range(n):
                        t = slice(2 * p, 2 * p + 2)
                        first = wi == 0 and p == 0
                        last = wi == 2 and p == n - 1
                        if vmode:
                            nc.tensor.matmul(
                                ps, lhsT=xt[:, t, c0 : c0 + 128],
                                rhs=w_sb[:, half, t, :],
                                start=first, stop=last, perf_mode=DR,
                            )
                        else:
                            nc.tensor.matmul(
                                ps, lhsT=w_sb[:, half, t, :], rhs=xt[:, t, :],
                                start=first, stop=last, perf_mode=DR,
                            )

            def k_unit(s0, w, xht, xlt):
                ps = pp.tile([128, w], f32, tag="ps_proj", name="ps")
                s3_matmuls(ps, wk_sb, xht, xlt)
                # no K bias: a per-query constant in the scores cancels
                nc.scalar.activation(kt_sb[:, s0 : s0 + w], ps, AF.Identity,
                                     scale=ISC)

            def q_unit(s0, w, xht, xlt, h):
                ps = pp.tile([128, w], f32, tag="ps_proj", name="ps")
                s3_matmuls(ps, wq_h[h], xht, xlt)
                # on DVE, not Act: an Act copy between exps delays the
                # score-psum ring in phase B
                nc.vector.scalar_tensor_tensor(
                    qt_sb[:, h, s0 : s0 + w], ps, ISC,
                    bq_sb[:, h : h + 1].broadcast_to((128, w)),
                    op0=ALU.mult, op1=ALU.add,
                )

            def v_unit(kj, s0, xht, xlt):
                # x is stationary (psum in [kpos, hd] layout); v then split
                # to an exact fp8 hi/lo pair for the DoubleRow AV matmul
                psv = pp.tile([128, HD], f32, tag="ps_proj", name="psv")
                s3_matmuls(psv, wv_sb, xht, xlt, c0=kj * 128 - s0, vmode=True)
                v16 = tpool.tile([128, HD], f16, tag="v16")
                nc.vector.scalar_tensor_tensor(
                    v16, psv, ISCV, bvb_sb, op0=ALU.mult, op1=ALU.add
                )
                nc.gpsimd.tensor_copy(v8_sb[:, kj, 0, :], v16)
                nc.gpsimd.tensor_tensor(
                    v8_sb[:, kj, 1, :], v16, v8_sb[:, kj, 0, :],
                    op=ALU.subtract,
                )

            NCH = len(CHUNKS)

            # --- attention slot pieces ---
            def score_pair(attn, h, q0, ktp):
                pss = psc.tile([128, 2, QC], f32, tag="ps_sc")
                for j in range(2):
                    kt = 2 * ktp + j
                    nc.tensor.matmul(
                        pss[:, j, :],
                        lhsT=kt_sb[:, kt * 128 : (kt + 1) * 128],
                        rhs=qt_sb[:, h, q0 : q0 + QC],
                        start=True,
                        stop=True,
                    )
                nc.scalar.activation(
                    attn[:, 2 * ktp : 2 * ktp + 2, :],
                    pss,
                    AF.Exp,
                    scale=INV_SQRT_HD,
                    bias=lnas_sb[:, 0:1],
                )

            def slot_post(attn, otc8, h, fillers, last=False):
                # denominator: fp8-ones DoubleRow matmul (matches the
                # quantized numerator exactly); AV DoubleRow with the exact
                # v hi/lo pair; both on the PE
                psd = po.tile([128, QC], f32, tag="ps_d", name="psd")
                for tp in range(KJT // 2):
                    nc.tensor.matmul(
                        psd, lhsT=ones8,
                        rhs=attn[:, 2 * tp : 2 * tp + 2, :],
                        start=(tp == 0), stop=(tp == KJT // 2 - 1),
                        perf_mode=DR,
                    )
                pso = po.tile([128, QC], f32, tag="ps_o")
                for kt in range(KJT):
                    rhs = attn[:, kt, :].unsqueeze(1).broadcast_to((128, 2, QC))
                    nc.tensor.matmul(
                        pso, lhsT=v8_sb[:, kt, :, :], rhs=rhs,
                        start=(kt == 0), stop=(kt == KJT - 1),
                        perf_mode=DR,
                    )
                rec = spool.tile([128, QC], f32, tag="rec")
                nc.vector.reciprocal(rec, psd)
                while fillers:
                    fillers.pop(0)()
                otc16 = tpool.tile([128, QC], f16, tag="otc16")
                nc.vector.tensor_tensor(otc16, pso, rec, op=ALU.mult)
                # DVE is faster; use it for the last slot where the otc
                # chain is on the critical path to the final Wo wave
                eng = nc.vector if last else nc.gpsimd
                eng.tensor_copy(otc8[:, h, 0, :], otc16)
                eng.tensor_tensor(
                    otc8[:, h, 1, :], otc16, otc8[:, h, 0, :],
                    op=ALU.subtract,
                )

            # --- Phase A: K/V projections chunk-streamed, with slot
            # (h0, qc0)'s score pairs folded in as kt slices land (score
            # pair ktp only needs kt up to chunk ktp+1) ---
            otc8_q = {}
            attn0 = apool.tile([128, KJT, QC], e4, tag="attn", name="attn0")
            attn1 = apool.tile([128, KJT, QC], e4, tag="attn", name="attn1")
            # x-independent extras per iteration: (head, chunk) Q units,
            # placed after the wq DMAs land
            EXTRA_Q = {
                3: [(0, 0)], 4: [(0, 1)], 5: [(0, 2), (1, 0)],
                6: [(1, 1), (2, 0)], 7: [(1, 2), (2, 1)], 8: [(2, 2)],
            }
            next_pair = 0
            for ci, (s0, w) in enumerate(CHUNKS):
                if ci not in xt_t:
                    xdma(ci)
                if ci == 8:
                    # h1's first 7 score pairs are x8-independent: they
                    # cover the x7/x8 DMA tail
                    for p in range(KJT // 2 - 1):
                        score_pair(attn1, 1, 0, p)
                k_unit(s0, w, xht=xt_t[ci][0], xlt=xt_t[ci][1])
                # V(0) lags one iteration: its wv DMA lands after x1
                if ci != 0:
                    vl = [(ci, s0)] if ci != 1 else [(0, 0), (1, CHUNKS[1][0])]
                    for vci, vs0 in vl:
                        vw = CHUNKS[vci][1]
                        for kj in range(vs0 // 128, (vs0 + vw) // 128):
                            v_unit(kj, vs0, xt_t[vci][0], xt_t[vci][1])
                for qh, qci in EXTRA_Q.get(ci, []):
                    qs0, qw = CHUNKS[qci]
                    q_unit(qs0, qw, xt_t[qci][0], xt_t[qci][1], qh)
                if ci >= 5:
                    while next_pair < KJT // 2 and next_pair + 1 <= ci:
                        score_pair(attn0, 0, 0, next_pair)
                        next_pair += 1
            nc.sync.dma_start(out=wo_sb, in_=wo_d)
            score_pair(attn1, 1, 0, KJT // 2 - 1)
            otc8_0 = opool.tile([128, HLOC, 2, QC], e4, tag="otc8")
            otc8_q[0] = otc8_0
            slot_post(attn0, otc8_0, 0, [])
            slot_post(attn1, otc8_0, 1, [])

            # --- Phase B: remaining slots + Wo interleave ---
            def wo_unit(qc, ec, otc8, copy_act=False, alt_pool=0):
                if alt_pool == 1:
                    psy = po.tile([128, QC], f32, tag="ps_o", name="psy")
                elif alt_pool == 2:
                    psy = po.tile([128, QC], f32, tag="ps_d", name="psy")
                else:
                    psy = pp.tile([128, QC], f32, tag="ps_proj", name="psy")
                esl = slice(ec * 128, (ec + 1) * 128)
                for h in range(HLOC):
                    rhs = otc8[:, h, 0, :].unsqueeze(1).broadcast_to((128, 2, QC))
                    nc.tensor.matmul(psy, lhsT=wo_sb[:, h, :, esl], rhs=rhs,
                                     start=(h == 0), stop=False, perf_mode=DR)
                for j in range(2):
                    nc.tensor.matmul(
                        psy,
                        lhsT=wo_sb[:, 2 * j : 2 * j + 2, 0, esl],
                        rhs=otc8[:, 2 * j : 2 * j + 2, 1, :],
                        start=False, stop=(j == 1), perf_mode=DR,
                    )
                ybuf = ypool.tile([128, QC], bf16, tag="yb", name="ybuf")
                # gpsimd can't read PSUM; DVE does the y copies (Act is
                # saturated by exp in phase B, but idle in the final wave)
                if copy_act:
                    nc.scalar.copy(ybuf, psy)
                else:
                    nc.vector.tensor_copy(ybuf, psy)
                nc.sync.dma_start(
                    out=yT[esl, qc * QC : (qc + 1) * QC], in_=ybuf
                )

            def q_filler(h, ci):
                s0, w = CHUNKS[ci]
                return lambda: q_unit(s0, w, xt_t[ci][0], xt_t[ci][1], h)

            # per-slot PE filler plan; Q(h, chunks) for a q-chunk must land
            # in an earlier slot than the (h, qc) slot that reads it.
            # qc0 reads chunks 0-2, qc1: 3-4, qc2: 5-6, qc3: 7-8.
            FILLER_PLAN = {
                (0, 2): [(3, 0), (3, 1), (3, 2)],
                (0, 3): [(0, 3), (0, 4), (1, 3), (1, 4)],
                (1, 0): [(2, 3), (2, 4)],
                (1, 1): [(3, 3), (3, 4)],
                (1, 2): [(0, 5), (0, 6)],
                (1, 3): [(1, 5), (1, 6)],
                (2, 0): [(2, 5), (2, 6)],
                (2, 1): [(3, 5), (3, 6)],
                (2, 2): [(0, 7), (0, 8)],
                (2, 3): [(1, 7), (1, 8)],
                (3, 0): [(2, 7), (2, 8)],
                (3, 1): [(3, 7), (3, 8)],
            }
            for qc in range(NQC):
                q0 = qc * QC
                if qc == 0:
                    otc8 = otc8_q[0]
                else:
                    otc8 = opool.tile([128, HLOC, 2, QC], e4, tag="otc8")
                    otc8_q[qc % 2] = otc8
                for h in range(HLOC):
                    if qc == 0 and h <= 1:
                        continue  # folded into phase A
                    fillers = []
                    wo_f = []
                    if qc > 0:
                        for i in range(4):
                            wo_f.append(
                                lambda ec=h * 4 + i, o=otc8_q[(qc - 1) % 2]:
                                    wo_unit(qc - 1, ec, o)
                            )
                    q_f = [q_filler(qh, qci)
                           for qh, qci in FILLER_PLAN.get((qc, h), [])]
                    if h == 0:
                        # Q units first: the first wo filler would wait on
                        # the previous qc's otc chain still in flight
                        fillers = q_f + wo_f
                    else:
                        fillers = wo_f + q_f
                    attn = apool.tile([128, KJT, QC], e4, tag="attn")
                    for ktp in range(KJT // 2):
                        score_pair(attn, h, q0, ktp)
                        if ktp % 2 == 1 and fillers:
                            fillers.pop(0)()
                    slot_post(attn, otc8, h, fillers,
                              last=(qc == NQC - 1 and h == HLOC - 1))
            for ec in range(ECH):
                wo_unit(NQC - 1, ec, otc8_q[(NQC - 1) % 2],
                        copy_act=(ec % 2 == 1),
                        alt_pool=(0, 1, 0, 2)[ec % 4])
    nc.finalize()
    return nc


def _get_nc():
    if "nc" not in _CACHE:
        _CACHE["nc"] = _build()
    return _CACHE["nc"]


def _hilo(a, s):
    import ml_dtypes

    E4 = ml_dtypes.float8_e4m3
    scaled = np.asarray(a, np.float32) * s
    hi = scaled.astype(E4)
    lo = (scaled - hi.astype(np.float32)).astype(E4)
    return hi, lo


def _shard_inputs(x, Wq, bq, Wk, bk, Wv, bv, Wo, bo):
    xT = np.asarray(x).transpose(0, 2, 1).astype(np.float32)  # [B, E, S]

    def xchunks(a):  # [E, S] fp8 -> [128, ECH*S] chunk-major
        parts = [
            a[:, s0 : s0 + w]
            .reshape(ECH, 128, w)
            .transpose(1, 0, 2)
            .reshape(128, ECH * w)
            for s0, w in CHUNKS
        ]
        return np.ascontiguousarray(np.concatenate(parts, axis=1))

    xcs = []
    for b in range(B):
        hi, lo = _hilo(xT[b], XS)
        xcs.append((xchunks(hi), xchunks(lo)))

    in_maps = []
    for d in range(NCORES):
        b = d // 4
        g = d % 4
        qh, ql = _hilo(Wq[:, g * 512 : (g + 1) * 512], WS)  # [E, 512]

        def wq_lay(a):
            return a.reshape(ECH, 128, HLOC, HD).transpose(2, 1, 0, 3)

        kh, kl = _hilo(Wk[:, g * 128 : (g + 1) * 128], WS)
        vh, vl = _hilo(Wv[:, g * 128 : (g + 1) * 128], WS)

        def wkv_lay(a):
            return a.reshape(ECH, 128, HD).transpose(1, 0, 2)

        oh, ol = _hilo(Wo[g * 512 : (g + 1) * 512, :], WS)  # [512, E]

        def wo_lay(a):
            return a.reshape(HLOC, 128, E).transpose(1, 0, 2)

        in_maps.append(
            {
                "xh": xcs[b][0],
                "xl": xcs[b][1],
                "wq": np.ascontiguousarray(
                    np.stack([wq_lay(qh), wq_lay(ql)], axis=2)
                ),
                "bq": np.ascontiguousarray(np.asarray(bq)[g * 512 : (g + 1) * 512]),
                "wk": np.ascontiguousarray(
                    np.stack([wkv_lay(kh), wkv_lay(kl)], axis=1)
                ),
                "wv": np.ascontiguousarray(
                    np.stack([wkv_lay(vh), wkv_lay(vl)], axis=1)
                ),
                "bvb": np.ascontiguousarray(
                    np.tile(
                        (np.asarray(bv)[g * 128 : (g + 1) * 128] * VS)[None, :],
                        (128, 1),
                    )
                ).astype(np.float16),
                "wo": np.ascontiguousarray(
                    np.stack([wo_lay(oh), wo_lay(ol)], axis=2)
                ),
            }
        )
    return in_maps


def _unshard(results, bo):
    acc = np.zeros((B, E, S), dtype=np.float32)
    for d, r in enumerate(results):
        acc[d // 4] += r["yT"].astype(np.float32)
    y = acc.transpose(0, 2, 1) / YDS + bo[None, None, :]
    return np.ascontiguousarray(y.astype(np.float32))


def kernel(x, Wq, bq, Wk, bk, Wv, bv, Wo, bo, **_):
    from concourse.bass_utils import run_bass_kernel_spmd

    nc = _get_nc()
    in_maps = _shard_inputs(x, Wq, bq, Wk, bk, Wv, bv, Wo, bo)
    res = run_bass_kernel_spmd(nc, in_maps, list(range(NCORES)))
    return _unshard(res.results, np.asarray(bo))
